# revision 47
# baseline (speedup 1.0000x reference)
"""Trainium2 Bass kernel for nn_AlignmentLoss (triplet + CE over phrase/input embeddings).

Sharding: batch dimension N=128 split 16 batches/core across 8 cores.  Each core
owns the positive pairs whose batch_idxs falls in its range (host buckets pairs,
padded to a fixed per-batch capacity).  All cosine distances are entries of the
similarity rows, so per pair the device computes: the sim row against its batch's
K inputs (PE matmul), top-8 row values (single DVE max8 instruction), s_pos and
the two random-negative sims (fused multiply+accumulate dots), and a CE
log-sum-exp (PE matmul + fused Exp/accum).  Hard-negative top-3 with the positive
masked out is recovered from unmasked top-4 values via
    sum_masked = sum_{i<=4} relu(t_i - s_pos + 1) - max(1, relu(t4 - s_pos + 1))
which needs no indexed masking.  Per-core partial sums are summed on host.

Engine split: PE does norm2 reductions (selector matmuls over GpSimd-squared
chunks), rows/CE matmuls and transposes; ACT does Square+accum norms, fused
rsqrt (Abs_reciprocal_sqrt) and Exp+accum; DVE does normalize-muls, dots,
rows-scale and max8.  Rand-negative sims dot the raw vectors and scale the
scalar afterwards, skipping 16 normalize passes.
"""

import sys

for _p in ("/opt/trn_rl_repo", "/root/.axon_site/_ro/trn_rl_repo"):
    if _p not in sys.path:
        sys.path.append(_p)

import numpy as np

import concourse.bass as bass
import concourse.bacc as bacc
import concourse.mybir as mybir
from concourse.tile import TileContext
from concourse.bass_utils import run_bass_kernel_spmd

F32 = mybir.dt.float32
BF16 = mybir.dt.bfloat16
AF = mybir.ActivationFunctionType
ALU = mybir.AluOpType
AX = mybir.AxisListType

N, K, M, D, P = 128, 1024, 512, 128, 4096
NCORES = 8
NB = N // NCORES  # batches per core = 16


def _bcast_free(ap, reps):
    """Append a 0-stride innermost free dim: (.., F) -> (.., F, reps)."""
    return bass.AP(tensor=ap.tensor, offset=ap.offset,
                   ap=list(ap.ap) + [[0, reps]])


def _bcast_parts(ap, nparts):
    """0-stride partition broadcast of a (1, F) DRAM AP for DMA."""
    return bass.AP(tensor=ap.tensor, offset=ap.offset,
                   ap=[[0, nparts]] + list(ap.ap[1:]))


def build_graph(cap: int, T: float) -> bass.Bass:
    """One-core SPMD graph; cap = padded pairs per batch; T = temperature."""
    C = NB * cap          # padded pairs per core
    NT = C // 128         # 128-pair tiles
    NBP = NB // 2         # batch-pairs (two batches share a 128-partition tile)
    assert cap % 64 == 0 and NT == NBP

    nc = bacc.Bacc(None, target_bir_lowering=False, debug=False)

    xt = nc.declare_dram_parameter("xt", [D, NB * K], BF16, isOutput=False)
    phr = nc.declare_dram_parameter("phr", [M, D], F32, isOutput=False)
    anc = nc.declare_dram_parameter("anc", [C, D], F32, isOutput=False)
    pos = nc.declare_dram_parameter("pos", [C, D], F32, isOutput=False)
    rng = nc.declare_dram_parameter("rng", [2 * C, D], F32, isOutput=False)
    vld = nc.declare_dram_parameter("vld", [128, NT], F32, isOutput=False)
    eye = nc.declare_dram_parameter("eye", [128, 128], F32, isOutput=False)
    sel4s = nc.declare_dram_parameter("sel4s", [128, 16], BF16, isOutput=False)
    out = nc.declare_dram_parameter("out", [16, 1], F32, isOutput=True)

    with TileContext(nc) as tc:
        with (
            tc.tile_pool(name="big", bufs=1) as big,
            tc.tile_pool(name="chunks", bufs=3) as chunks,
            tc.tile_pool(name="work", bufs=4) as work,
            tc.tile_pool(name="small", bufs=8) as small,
            tc.tile_pool(name="rowsp", bufs=2) as rowsbp,
            tc.tile_pool(name="dram", bufs=1, space="DRAM") as dram,
            tc.tile_pool(name="pn2", bufs=2, space="PSUM") as pn2,
            tc.tile_pool(name="prows", bufs=2, space="PSUM") as prows,
            tc.tile_pool(name="psmall", bufs=2, space="PSUM") as psmall,
        ):
            # ---- constants / small inputs ----
            eye_sb = big.tile([128, 128], F32, tag="eye")
            nc.sync.dma_start(out=eye_sb, in_=eye[:, :])
            sel4s_sb = big.tile([128, 16], BF16, tag="sel4s")
            nc.sync.dma_start(out=sel4s_sb, in_=sel4s[:, :])
            vld_sb = big.tile([128, NT], F32, tag="vld")
            nc.sync.dma_start(out=vld_sb, in_=vld[:, :])
            ones_col = big.tile([128, 1], F32, tag="ones")
            nc.vector.memset(ones_col, 1.0)

            # big persistent tensors
            xt_sb = big.tile([128, NB * K], BF16, tag="xt")
            anchatT = big.tile([128, C], BF16, tag="anchatT")
            poshatT = big.tile([128, C], BF16, tag="poshatT")
            phatT = big.tile([128, M], BF16, tag="phatT")
            rinv32 = big.tile([4, NBP * 512], BF16, tag="rinv32")
            rinv_dr = dram.tile([2, NBP * 1024], BF16, tag="rinvdr")
            rnb_all = big.tile([128, NBP * K], BF16, tag="rnball")
            t8_all = big.tile([128, NT * 8], F32, tag="t8")
            spos = big.tile([128, NT], F32, tag="spos")
            srnd = big.tile([128, 2 * NT], F32, tag="srnd")
            sumexp = big.tile([128, NT], F32, tag="sumexp")
            stat = big.tile([128, 2 * NT], F32, tag="stat")

            # normalize working set: [anc | pos | phr | rng] row blocks
            NPH = M // 128
            NTOT = 2 * NT + NPH + 2 * NT
            IANC, IPOS, IPHR, IRNG = 0, NT, 2 * NT, 2 * NT + NPH
            srcs = [anc[t * 128:(t + 1) * 128, :] for t in range(NT)]
            srcs += [pos[t * 128:(t + 1) * 128, :] for t in range(NT)]
            srcs += [phr[t * 128:(t + 1) * 128, :] for t in range(NPH)]
            srcs += [rng[t * 128:(t + 1) * 128, :] for t in range(2 * NT)]
            xall = big.tile([128, NTOT * 128], F32, tag="xall")
            n2cols = big.tile([128, NTOT], F32, tag="n2cols")
            rinvc = big.tile([128, NTOT], F32, tag="rinvc")

            def seg_norms(i0, i1):
                """DMA + Square/accum for tiles [i0,i1), then one rsqrt."""
                for i in range(i0, i1):
                    sl = xall[:, i * 128:(i + 1) * 128]
                    nc.sync.dma_start(out=sl, in_=srcs[i])
                    junk = work.tile([128, D], F32, tag="junk")
                    nc.scalar.activation(junk, sl, AF.Square,
                                         accum_out=n2cols[:, i:i + 1])
                nc.scalar.activation(rinvc[:, i0:i1], n2cols[:, i0:i1],
                                     AF.Abs_reciprocal_sqrt)

            def norm_slice(dst, i):
                nc.vector.tensor_scalar_mul(
                    dst, xall[:, i * 128:(i + 1) * 128], rinvc[:, i:i + 1])
                return dst

            def transpose_to(dstT_slice, src_tile):
                ps = psmall.tile([128, 512], F32, tag="pm")
                nc.tensor.transpose(ps[:, :128], src_tile, eye_sb)
                nc.vector.tensor_copy(dstT_slice, ps[:, :128])

            # ---- anchors/positives transposed RAW (norms folded into the
            # rows-scale per-partition scalar and the Exp per-partition scale)
            seg_norms(IANC, IANC + NT)
            for t in range(NT):
                transpose_to(anchatT[:, t * 128:(t + 1) * 128],
                             xall[:, (IANC + t) * 128:(IANC + t + 1) * 128])
            seg_norms(IPOS, IPOS + NT)
            for t in range(NT):
                po = work.tile([128, D], F32, tag="pon")
                norm_slice(po, IPOS + t)
                transpose_to(poshatT[:, t * 128:(t + 1) * 128], po)
                junk2 = work.tile([128, D], F32, tag="junk2")
                nc.vector.scalar_tensor_tensor(
                    junk2, xall[:, (IANC + t) * 128:(IANC + t + 1) * 128],
                    1.0, po, op0=ALU.mult, op1=ALU.mult,
                    accum_out=spos[:, t:t + 1],
                )
            # s_pos = (raw anchor . pos_hat) * rinv_anc
            nc.vector.tensor_mul(spos, spos, rinvc[:, IANC:IANC + NT])
            seg_norms(IRNG, IRNG + 2 * NT)
            for r in range(2):
                for t in range(NT):
                    # raw x raw dot; scale the scalar afterwards
                    junk3 = work.tile([128, D], F32, tag="junk3")
                    i = IRNG + r * NT + t
                    nc.vector.scalar_tensor_tensor(
                        junk3, xall[:, (IANC + t) * 128:(IANC + t + 1) * 128],
                        1.0, xall[:, i * 128:(i + 1) * 128],
                        op0=ALU.mult, op1=ALU.mult,
                        accum_out=srnd[:, r * NT + t:r * NT + t + 1],
                    )
            for r in range(2):
                nc.vector.tensor_mul(srnd[:, r * NT:(r + 1) * NT],
                                     srnd[:, r * NT:(r + 1) * NT],
                                     rinvc[:, IANC:IANC + NT])
            nc.vector.tensor_mul(srnd, srnd, rinvc[:, IRNG:IRNG + 2 * NT])
            seg_norms(IPHR, IPHR + NPH)
            for t in range(NPH):
                ph = work.tile([128, D], F32, tag="phn")
                norm_slice(ph, IPHR + t)
                transpose_to(phatT[:, t * 128:(t + 1) * 128], ph)

            # ---- merged per-batch-pair pipeline: input norms -> rnb -> rows
            # -> top8, so the PE stream interleaves and the tail follows the
            # square chain instead of stacking phases ----
            for bp in range(NBP):
                n2w = pn2.tile([4, 512], F32, tag="n2w")
                for g in range(2):
                    # one kilo-wide DMA + square per chunk pair (amortizes
                    # per-instruction overhead on the pacing GpSimd chain)
                    c0 = 4 * bp + 2 * g
                    sl2 = xt_sb[:, c0 * 512:(c0 + 2) * 512]
                    nc.sync.dma_start(
                        out=sl2, in_=xt[:, c0 * 512:(c0 + 2) * 512])
                    sq = chunks.tile([128, 1024], BF16, tag="sq")
                    nc.gpsimd.tensor_mul(sq, sl2, sl2)
                    for j in range(2):
                        cc = 2 * g + j
                        nc.tensor.matmul(
                            n2w, sel4s_sb[:, 4 * cc:4 * cc + 4],
                            sq[:, j * 512:(j + 1) * 512],
                            start=(cc == 0), stop=(cc == 3))
                nc.scalar.activation(
                    rinv32[:, bp * 512:(bp + 1) * 512], n2w,
                    AF.Abs_reciprocal_sqrt)
                nc.sync.dma_start(
                    out=rinv_dr[:, bp * 1024:(bp + 1) * 1024],
                    in_=rinv32[:, bp * 512:(bp + 1) * 512])
                for h in range(2):
                    src = rinv_dr[h:h + 1, bp * 1024:(bp + 1) * 1024]
                    nc.sync.dma_start(
                        out=rnb_all[64 * h:64 * h + 64, bp * K:(bp + 1) * K],
                        in_=_bcast_parts(src, 64))

                rnb_sb = rnb_all[:, bp * K:(bp + 1) * K]
                rows_sb = rowsbp.tile([128, K], F32, tag="rows")
                rp = prows.tile([128, K], F32, tag="rp")
                for half in range(2):
                    b = 2 * bp + half
                    for h in range(2):
                        nc.tensor.matmul(
                            rp[half * 64:(half + 1) * 64,
                               h * 512:(h + 1) * 512],
                            anchatT[:, b * cap:b * cap + cap],
                            xt_sb[:, b * K + h * 512:b * K + (h + 1) * 512],
                            start=True, stop=True)
                nc.vector.scalar_tensor_tensor(
                    rows_sb, rp, rinvc[:, IANC + bp:IANC + bp + 1], rnb_sb,
                    op0=ALU.mult, op1=ALU.mult)
                nc.vector.max(t8_all[:, bp * 8:(bp + 1) * 8], rows_sb)

                if bp == 1:
                    # CE: logits + exp/accum per pair tile (bf16 matmuls);
                    # inputs are ready by now, keeps the tail free
                    for t in range(NT):
                        lg = psmall.tile([128, 512], F32, tag="pm")
                        nc.tensor.matmul(
                            lg, poshatT[:, t * 128:(t + 1) * 128], phatT,
                            start=True, stop=True)
                        junk4 = work.tile([128, 512], F32, tag="junk4")
                        nc.scalar.activation(
                            junk4, lg, AF.Exp, scale=float(T),
                            accum_out=sumexp[:, t:t + 1])


            # ---- finale, batched over all NT pair tiles ----
            t83 = t8_all[:, :].rearrange("p (t e) -> p t e", e=8)
            u_all = big.tile([128, NT * 8], F32, tag="uall")
            u3 = u_all[:, :].rearrange("p (t e) -> p t e", e=8)
            nc.vector.scalar_tensor_tensor(
                u3, t83, 1.0, _bcast_free(spos[:, :], 8),
                op0=ALU.add, op1=ALU.subtract)
            nc.vector.tensor_scalar_max(u_all, u_all, 0.0)
            s4 = small.tile([128, NT], F32, tag="s4")
            nc.vector.tensor_reduce(s4, u3[:, :, 0:4], AX.X, ALU.add)
            w = small.tile([128, NT], F32, tag="w")
            u4th = u_all[:, 3:4]
            u4th = bass.AP(tensor=u4th.tensor, offset=u4th.offset,
                           ap=[u4th.ap[0], [8, NT]])
            nc.vector.tensor_scalar_max(w, u4th, 1.0)
            hard = small.tile([128, NT], F32, tag="hard")
            nc.vector.tensor_sub(hard, s4, w)
            # random negatives: srnd is [r*NT + t] column order
            ur = small.tile([128, 2 * NT], F32, tag="ur")
            ur3 = ur[:, :].rearrange("p (t r) -> p t r", r=2)
            nc.vector.scalar_tensor_tensor(
                ur3, srnd[:, :].rearrange("p (r t) -> p t r", r=2), 1.0,
                _bcast_free(spos[:, :], 2),
                op0=ALU.add, op1=ALU.subtract)
            nc.vector.tensor_scalar_max(ur, ur, 0.0)
            r2 = small.tile([128, NT], F32, tag="r2")
            nc.vector.tensor_reduce(r2, ur3, AX.X, ALU.add)
            tript = small.tile([128, NT], F32, tag="tript")
            nc.vector.tensor_add(tript, hard, r2)
            nc.vector.tensor_mul(stat[:, 0:NT], tript, vld_sb)
            lnse = small.tile([128, NT], F32, tag="lnse")
            nc.scalar.activation(lnse, sumexp, AF.Ln)
            tsp = small.tile([128, NT], F32, tag="tsp")
            nc.vector.tensor_scalar_mul(tsp, spos, float(T))
            cet = small.tile([128, NT], F32, tag="cet")
            nc.vector.tensor_sub(cet, lnse, tsp)
            nc.vector.tensor_mul(stat[:, NT:2 * NT], cet, vld_sb)

            # ---- cross-partition reduction: out[j] = sum_p stat[p, j] ----
            pres = psmall.tile([128, 512], F32, tag="pm")
            nc.tensor.matmul(
                pres[:2 * NT, :1], stat, ones_col, start=True, stop=True)
            res_sb = small.tile([2 * NT, 1], F32, tag="res")
            nc.vector.tensor_copy(res_sb, pres[:2 * NT, :1])
            nc.sync.dma_start(out=out[:, :], in_=res_sb[:, :])

    if not nc.is_finalized():
        nc.finalize()
    return nc


_CACHE = {}


def _prep_core(c, cap, pe, ie, bi, mi, ki, rn, T):
    C = NB * cap
    NT = C // 128
    lo = NB * c
    sel = np.where((bi >= lo) & (bi < lo + NB))[0]
    # pad with unit vectors so normalization never divides by zero
    ancb = np.zeros((C, D), np.float32); ancb[:, 0] = 1.0
    posb = np.zeros((C, D), np.float32); posb[:, 0] = 1.0
    rngb = np.zeros((2 * C, D), np.float32); rngb[:, 0] = 1.0
    valid = np.zeros(C, np.float32)
    for n in range(NB):
        pb = sel[bi[sel] == lo + n]
        assert len(pb) <= cap
        s = n * cap
        ancb[s:s + len(pb)] = pe[mi[pb]]
        posb[s:s + len(pb)] = ie[bi[pb], ki[pb]]
        rngb[s:s + len(pb)] = ie[bi[pb], rn[pb, 0]]
        rngb[C + s:C + s + len(pb)] = ie[bi[pb], rn[pb, 1]]
        valid[s:s + len(pb)] = 1.0
    xt_c = np.ascontiguousarray(
        ie[lo:lo + NB].reshape(NB * K, D).T).astype(mybir.dt.np(BF16))
    vld_dev = np.ascontiguousarray(valid.reshape(NT, 128).T)
    sel4s = np.zeros((128, 16), mybir.dt.np(BF16))
    for j in range(4):
        sel4s[:, 4 * j + j] = 1.0
    return dict(
        xt=xt_c, phr=pe, anc=ancb, pos=posb, rng=rngb, vld=vld_dev,
        eye=np.eye(128, dtype=np.float32),
        sel4s=sel4s,
    )


def make_in_maps(inputs, cap=None):
    pe = np.asarray(inputs["phrase_embeddings"], np.float32)
    ie = np.asarray(inputs["input_embeddings"], np.float32)
    bi = np.asarray(inputs["batch_idxs"])
    mi = np.asarray(inputs["phrase_emb_idxs"])
    ki = np.asarray(inputs["input_emb_idxs"])
    rn = np.asarray(inputs["rand_neg_idx"])
    T = float(np.asarray(inputs["temperature"]))
    if cap is None:
        maxc = int(np.bincount(bi, minlength=N).max())
        cap = max(64, ((maxc + 63) // 64) * 64)
    return [
        _prep_core(c, cap, pe, ie, bi, mi, ki, rn, T) for c in range(NCORES)
    ], cap, T


def kernel(**inputs):
    in_maps, cap, T = make_in_maps(inputs)
    key = (cap, T)
    if key not in _CACHE:
        _CACHE[key] = build_graph(cap, T)
    nc = _CACHE[key]
    res = run_bass_kernel_spmd(nc, in_maps, core_ids=list(range(NCORES)))
    outs = np.stack([np.asarray(r["out"]).reshape(-1) for r in res.results])
    NT = NB * cap // 128
    trip = outs[:, :NT].sum() / (P * 5)
    ce = outs[:, NT:].sum() / P
    return np.float32(trip), np.float32(ce)


# revision 48
# speedup vs baseline: 1.0352x; 1.0352x over previous
"""Trainium2 Bass kernel for nn_AlignmentLoss (triplet + CE over phrase/input embeddings).

Sharding: batch dimension N=128 split 16 batches/core across 8 cores.  Each core
owns the positive pairs whose batch_idxs falls in its range (host buckets pairs,
padded to a fixed per-batch capacity).  All cosine distances are entries of the
similarity rows, so per pair the device computes: the sim row against its batch's
K inputs (PE matmul), top-8 row values (single DVE max8 instruction), s_pos and
the two random-negative sims (fused multiply+accumulate dots), and a CE
log-sum-exp (PE matmul + fused Exp/accum).  Hard-negative top-3 with the positive
masked out is recovered from unmasked top-4 values via
    sum_masked = sum_{i<=4} relu(t_i - s_pos + 1) - max(1, relu(t4 - s_pos + 1))
which needs no indexed masking.  Per-core partial sums are summed on host.

Engine split: PE does norm2 reductions (selector matmuls over GpSimd-squared
chunks), rows/CE matmuls and transposes; ACT does Square+accum norms, fused
rsqrt (Abs_reciprocal_sqrt) and Exp+accum; DVE does normalize-muls, dots,
rows-scale and max8.  Rand-negative sims dot the raw vectors and scale the
scalar afterwards, skipping 16 normalize passes.
"""

import sys

for _p in ("/opt/trn_rl_repo", "/root/.axon_site/_ro/trn_rl_repo"):
    if _p not in sys.path:
        sys.path.append(_p)

import numpy as np

import concourse.bass as bass
import concourse.bacc as bacc
import concourse.mybir as mybir
from concourse.tile import TileContext
from concourse.bass_utils import run_bass_kernel_spmd

F32 = mybir.dt.float32
BF16 = mybir.dt.bfloat16
AF = mybir.ActivationFunctionType
ALU = mybir.AluOpType
AX = mybir.AxisListType

N, K, M, D, P = 128, 1024, 512, 128, 4096
NCORES = 8
NB = N // NCORES  # batches per core = 16


def _bcast_free(ap, reps):
    """Append a 0-stride innermost free dim: (.., F) -> (.., F, reps)."""
    return bass.AP(tensor=ap.tensor, offset=ap.offset,
                   ap=list(ap.ap) + [[0, reps]])


def _bcast_parts(ap, nparts):
    """0-stride partition broadcast of a (1, F) DRAM AP for DMA."""
    return bass.AP(tensor=ap.tensor, offset=ap.offset,
                   ap=[[0, nparts]] + list(ap.ap[1:]))


def build_graph(cap: int, T: float) -> bass.Bass:
    """One-core SPMD graph; cap = padded pairs per batch; T = temperature."""
    C = NB * cap          # padded pairs per core
    NT = C // 128         # 128-pair tiles
    NBP = NB // 2         # batch-pairs (two batches share a 128-partition tile)
    assert cap % 64 == 0 and NT == NBP

    nc = bacc.Bacc(None, target_bir_lowering=False, debug=False)

    xt = nc.declare_dram_parameter("xt", [D, NB * K], BF16, isOutput=False)
    phr = nc.declare_dram_parameter("phr", [M, D], F32, isOutput=False)
    anc = nc.declare_dram_parameter("anc", [C, D], F32, isOutput=False)
    pos = nc.declare_dram_parameter("pos", [C, D], F32, isOutput=False)
    rng = nc.declare_dram_parameter("rng", [2 * C, D], F32, isOutput=False)
    vld = nc.declare_dram_parameter("vld", [128, NT], F32, isOutput=False)
    eye = nc.declare_dram_parameter("eye", [128, 128], F32, isOutput=False)
    sel4s = nc.declare_dram_parameter("sel4s", [128, 16], BF16, isOutput=False)
    out = nc.declare_dram_parameter("out", [16, 1], F32, isOutput=True)

    with TileContext(nc) as tc:
        with (
            tc.tile_pool(name="big", bufs=1) as big,
            tc.tile_pool(name="chunks", bufs=3) as chunks,
            tc.tile_pool(name="work", bufs=4) as work,
            tc.tile_pool(name="small", bufs=8) as small,
            tc.tile_pool(name="rowsp", bufs=2) as rowsbp,
            tc.tile_pool(name="dram", bufs=1, space="DRAM") as dram,
            tc.tile_pool(name="pn2", bufs=2, space="PSUM") as pn2,
            tc.tile_pool(name="prows", bufs=2, space="PSUM") as prows,
            tc.tile_pool(name="psmall", bufs=2, space="PSUM") as psmall,
        ):
            # ---- constants / small inputs ----
            eye_sb = big.tile([128, 128], F32, tag="eye")
            nc.sync.dma_start(out=eye_sb, in_=eye[:, :])
            sel4s_sb = big.tile([128, 16], BF16, tag="sel4s")
            nc.sync.dma_start(out=sel4s_sb, in_=sel4s[:, :])
            vld_sb = big.tile([128, NT], F32, tag="vld")
            nc.sync.dma_start(out=vld_sb, in_=vld[:, :])
            ones_col = big.tile([128, 1], F32, tag="ones")
            nc.vector.memset(ones_col, 1.0)

            # big persistent tensors
            xt_sb = big.tile([128, NB * K], BF16, tag="xt")
            anchatT = big.tile([128, C], BF16, tag="anchatT")
            poshatT = big.tile([128, C], BF16, tag="poshatT")
            phatT = big.tile([128, M], BF16, tag="phatT")
            rinv32 = big.tile([4, NBP * 512], BF16, tag="rinv32")
            rinv_dr = dram.tile([2, NBP * 1024], BF16, tag="rinvdr")
            rnb_all = big.tile([128, NBP * K], BF16, tag="rnball")
            t8_all = big.tile([128, NT * 8], F32, tag="t8")
            spos = big.tile([128, NT], F32, tag="spos")
            srnd = big.tile([128, 2 * NT], F32, tag="srnd")
            sumexp = big.tile([128, NT], F32, tag="sumexp")
            stat = big.tile([128, 2 * NT], F32, tag="stat")

            # normalize working set: [anc | pos | phr | rng] row blocks
            NPH = M // 128
            NTOT = 2 * NT + NPH + 2 * NT
            IANC, IPOS, IPHR, IRNG = 0, NT, 2 * NT, 2 * NT + NPH
            srcs = [anc[t * 128:(t + 1) * 128, :] for t in range(NT)]
            srcs += [pos[t * 128:(t + 1) * 128, :] for t in range(NT)]
            srcs += [phr[t * 128:(t + 1) * 128, :] for t in range(NPH)]
            srcs += [rng[t * 128:(t + 1) * 128, :] for t in range(2 * NT)]
            xall = big.tile([128, NTOT * 128], F32, tag="xall")
            n2cols = big.tile([128, NTOT], F32, tag="n2cols")
            rinvc = big.tile([128, NTOT], F32, tag="rinvc")

            def seg_norms(i0, i1):
                """DMA + Square/accum for tiles [i0,i1), then one rsqrt."""
                for i in range(i0, i1):
                    sl = xall[:, i * 128:(i + 1) * 128]
                    nc.sync.dma_start(out=sl, in_=srcs[i])
                    junk = work.tile([128, D], F32, tag="junk")
                    nc.scalar.activation(junk, sl, AF.Square,
                                         accum_out=n2cols[:, i:i + 1])
                nc.scalar.activation(rinvc[:, i0:i1], n2cols[:, i0:i1],
                                     AF.Abs_reciprocal_sqrt)

            def norm_slice(dst, i):
                nc.vector.tensor_scalar_mul(
                    dst, xall[:, i * 128:(i + 1) * 128], rinvc[:, i:i + 1])
                return dst

            def transpose_to(dstT_slice, src_tile):
                ps = psmall.tile([128, 512], F32, tag="pm")
                nc.tensor.transpose(ps[:, :128], src_tile, eye_sb)
                nc.vector.tensor_copy(dstT_slice, ps[:, :128])

            # ---- anchors/positives transposed RAW (norms folded into the
            # rows-scale per-partition scalar and the Exp per-partition scale)
            seg_norms(IANC, IANC + NT)
            for t in range(NT):
                transpose_to(anchatT[:, t * 128:(t + 1) * 128],
                             xall[:, (IANC + t) * 128:(IANC + t + 1) * 128])
            seg_norms(IPOS, IPOS + NT)
            for t in range(NT):
                po = work.tile([128, D], F32, tag="pon")
                norm_slice(po, IPOS + t)
                transpose_to(poshatT[:, t * 128:(t + 1) * 128], po)
                junk2 = work.tile([128, D], F32, tag="junk2")
                nc.vector.scalar_tensor_tensor(
                    junk2, xall[:, (IANC + t) * 128:(IANC + t + 1) * 128],
                    1.0, po, op0=ALU.mult, op1=ALU.mult,
                    accum_out=spos[:, t:t + 1],
                )
            # s_pos = (raw anchor . pos_hat) * rinv_anc
            nc.vector.tensor_mul(spos, spos, rinvc[:, IANC:IANC + NT])
            seg_norms(IRNG, IRNG + 2 * NT)
            for r in range(2):
                for t in range(NT):
                    # raw x raw dot; scale the scalar afterwards
                    junk3 = work.tile([128, D], F32, tag="junk3")
                    i = IRNG + r * NT + t
                    nc.vector.scalar_tensor_tensor(
                        junk3, xall[:, (IANC + t) * 128:(IANC + t + 1) * 128],
                        1.0, xall[:, i * 128:(i + 1) * 128],
                        op0=ALU.mult, op1=ALU.mult,
                        accum_out=srnd[:, r * NT + t:r * NT + t + 1],
                    )
            for r in range(2):
                nc.vector.tensor_mul(srnd[:, r * NT:(r + 1) * NT],
                                     srnd[:, r * NT:(r + 1) * NT],
                                     rinvc[:, IANC:IANC + NT])
            nc.vector.tensor_mul(srnd, srnd, rinvc[:, IRNG:IRNG + 2 * NT])
            seg_norms(IPHR, IPHR + NPH)
            for t in range(NPH):
                ph = work.tile([128, D], F32, tag="phn")
                norm_slice(ph, IPHR + t)
                transpose_to(phatT[:, t * 128:(t + 1) * 128], ph)

            # ---- merged per-batch-pair pipeline: input norms -> rnb -> rows
            # -> top8, so the PE stream interleaves and the tail follows the
            # square chain instead of stacking phases ----
            for bp in range(NBP):
                n2w = pn2.tile([4, 512], F32, tag="n2w")
                for g in range(2):
                    # one kilo-wide DMA + square per chunk pair (amortizes
                    # per-instruction overhead on the pacing GpSimd chain)
                    c0 = 4 * bp + 2 * g
                    sl2 = xt_sb[:, c0 * 512:(c0 + 2) * 512]
                    nc.sync.dma_start(
                        out=sl2, in_=xt[:, c0 * 512:(c0 + 2) * 512])
                    sq = chunks.tile([128, 1024], BF16, tag="sq")
                    nc.gpsimd.tensor_mul(sq, sl2, sl2)
                    for j in range(2):
                        cc = 2 * g + j
                        nc.tensor.matmul(
                            n2w, sel4s_sb[:, 4 * cc:4 * cc + 4],
                            sq[:, j * 512:(j + 1) * 512],
                            start=(cc == 0), stop=(cc == 3))
                nc.scalar.activation(
                    rinv32[:, bp * 512:(bp + 1) * 512], n2w,
                    AF.Abs_reciprocal_sqrt)
                nc.sync.dma_start(
                    out=rinv_dr[:, bp * 1024:(bp + 1) * 1024],
                    in_=rinv32[:, bp * 512:(bp + 1) * 512])
                for h in range(2):
                    src = rinv_dr[h:h + 1, bp * 1024:(bp + 1) * 1024]
                    nc.sync.dma_start(
                        out=rnb_all[64 * h:64 * h + 64, bp * K:(bp + 1) * K],
                        in_=_bcast_parts(src, 64))

                rnb_sb = rnb_all[:, bp * K:(bp + 1) * K]
                rows_sb = rowsbp.tile([128, K], F32, tag="rows")
                rp = prows.tile([128, K], F32, tag="rp")
                for half in range(2):
                    b = 2 * bp + half
                    for h in range(2):
                        nc.tensor.matmul(
                            rp[half * 64:(half + 1) * 64,
                               h * 512:(h + 1) * 512],
                            anchatT[:, b * cap:b * cap + cap],
                            xt_sb[:, b * K + h * 512:b * K + (h + 1) * 512],
                            start=True, stop=True)
                nc.vector.scalar_tensor_tensor(
                    rows_sb, rp, rinvc[:, IANC + bp:IANC + bp + 1], rnb_sb,
                    op0=ALU.mult, op1=ALU.mult)
                nc.vector.max(t8_all[:, bp * 8:(bp + 1) * 8], rows_sb)

                if bp == 3:
                    # CE: logits + exp/accum per pair tile (bf16 matmuls);
                    # inputs are ready by now, keeps the tail free
                    for t in range(NT):
                        lg = psmall.tile([128, 512], F32, tag="pm")
                        nc.tensor.matmul(
                            lg, poshatT[:, t * 128:(t + 1) * 128], phatT,
                            start=True, stop=True)
                        junk4 = work.tile([128, 512], F32, tag="junk4")
                        nc.scalar.activation(
                            junk4, lg, AF.Exp, scale=float(T),
                            accum_out=sumexp[:, t:t + 1])


            # ---- finale, batched over all NT pair tiles ----
            t83 = t8_all[:, :].rearrange("p (t e) -> p t e", e=8)
            u_all = big.tile([128, NT * 8], F32, tag="uall")
            u3 = u_all[:, :].rearrange("p (t e) -> p t e", e=8)
            nc.vector.scalar_tensor_tensor(
                u3, t83, 1.0, _bcast_free(spos[:, :], 8),
                op0=ALU.add, op1=ALU.subtract)
            nc.vector.tensor_scalar_max(u_all, u_all, 0.0)
            s4 = small.tile([128, NT], F32, tag="s4")
            nc.vector.tensor_reduce(s4, u3[:, :, 0:4], AX.X, ALU.add)
            w = small.tile([128, NT], F32, tag="w")
            u4th = u_all[:, 3:4]
            u4th = bass.AP(tensor=u4th.tensor, offset=u4th.offset,
                           ap=[u4th.ap[0], [8, NT]])
            nc.vector.tensor_scalar_max(w, u4th, 1.0)
            hard = small.tile([128, NT], F32, tag="hard")
            nc.vector.tensor_sub(hard, s4, w)
            # random negatives: srnd is [r*NT + t] column order
            ur = small.tile([128, 2 * NT], F32, tag="ur")
            ur3 = ur[:, :].rearrange("p (t r) -> p t r", r=2)
            nc.vector.scalar_tensor_tensor(
                ur3, srnd[:, :].rearrange("p (r t) -> p t r", r=2), 1.0,
                _bcast_free(spos[:, :], 2),
                op0=ALU.add, op1=ALU.subtract)
            nc.vector.tensor_scalar_max(ur, ur, 0.0)
            r2 = small.tile([128, NT], F32, tag="r2")
            nc.vector.tensor_reduce(r2, ur3, AX.X, ALU.add)
            tript = small.tile([128, NT], F32, tag="tript")
            nc.vector.tensor_add(tript, hard, r2)
            nc.vector.tensor_mul(stat[:, 0:NT], tript, vld_sb)
            lnse = small.tile([128, NT], F32, tag="lnse")
            nc.scalar.activation(lnse, sumexp, AF.Ln)
            tsp = small.tile([128, NT], F32, tag="tsp")
            nc.vector.tensor_scalar_mul(tsp, spos, float(T))
            cet = small.tile([128, NT], F32, tag="cet")
            nc.vector.tensor_sub(cet, lnse, tsp)
            nc.vector.tensor_mul(stat[:, NT:2 * NT], cet, vld_sb)

            # ---- cross-partition reduction: out[j] = sum_p stat[p, j] ----
            pres = psmall.tile([128, 512], F32, tag="pm")
            nc.tensor.matmul(
                pres[:2 * NT, :1], stat, ones_col, start=True, stop=True)
            res_sb = small.tile([2 * NT, 1], F32, tag="res")
            nc.vector.tensor_copy(res_sb, pres[:2 * NT, :1])
            nc.sync.dma_start(out=out[:, :], in_=res_sb[:, :])

    if not nc.is_finalized():
        nc.finalize()
    return nc


_CACHE = {}


def _prep_core(c, cap, pe, ie, bi, mi, ki, rn, T):
    C = NB * cap
    NT = C // 128
    lo = NB * c
    sel = np.where((bi >= lo) & (bi < lo + NB))[0]
    # pad with unit vectors so normalization never divides by zero
    ancb = np.zeros((C, D), np.float32); ancb[:, 0] = 1.0
    posb = np.zeros((C, D), np.float32); posb[:, 0] = 1.0
    rngb = np.zeros((2 * C, D), np.float32); rngb[:, 0] = 1.0
    valid = np.zeros(C, np.float32)
    for n in range(NB):
        pb = sel[bi[sel] == lo + n]
        assert len(pb) <= cap
        s = n * cap
        ancb[s:s + len(pb)] = pe[mi[pb]]
        posb[s:s + len(pb)] = ie[bi[pb], ki[pb]]
        rngb[s:s + len(pb)] = ie[bi[pb], rn[pb, 0]]
        rngb[C + s:C + s + len(pb)] = ie[bi[pb], rn[pb, 1]]
        valid[s:s + len(pb)] = 1.0
    xt_c = np.ascontiguousarray(
        ie[lo:lo + NB].reshape(NB * K, D).T).astype(mybir.dt.np(BF16))
    vld_dev = np.ascontiguousarray(valid.reshape(NT, 128).T)
    sel4s = np.zeros((128, 16), mybir.dt.np(BF16))
    for j in range(4):
        sel4s[:, 4 * j + j] = 1.0
    return dict(
        xt=xt_c, phr=pe, anc=ancb, pos=posb, rng=rngb, vld=vld_dev,
        eye=np.eye(128, dtype=np.float32),
        sel4s=sel4s,
    )


def make_in_maps(inputs, cap=None):
    pe = np.asarray(inputs["phrase_embeddings"], np.float32)
    ie = np.asarray(inputs["input_embeddings"], np.float32)
    bi = np.asarray(inputs["batch_idxs"])
    mi = np.asarray(inputs["phrase_emb_idxs"])
    ki = np.asarray(inputs["input_emb_idxs"])
    rn = np.asarray(inputs["rand_neg_idx"])
    T = float(np.asarray(inputs["temperature"]))
    if cap is None:
        maxc = int(np.bincount(bi, minlength=N).max())
        cap = max(64, ((maxc + 63) // 64) * 64)
    return [
        _prep_core(c, cap, pe, ie, bi, mi, ki, rn, T) for c in range(NCORES)
    ], cap, T


def kernel(**inputs):
    in_maps, cap, T = make_in_maps(inputs)
    key = (cap, T)
    if key not in _CACHE:
        _CACHE[key] = build_graph(cap, T)
    nc = _CACHE[key]
    res = run_bass_kernel_spmd(nc, in_maps, core_ids=list(range(NCORES)))
    outs = np.stack([np.asarray(r["out"]).reshape(-1) for r in res.results])
    NT = NB * cap // 128
    trip = outs[:, :NT].sum() / (P * 5)
    ce = outs[:, NT:].sum() / P
    return np.float32(trip), np.float32(ce)
